# revision 17
# baseline (speedup 1.0000x reference)
"""Causal self-attention (B=2, T=2048, C=1024, H=16) on 8 NeuronCores.

Sharding: data-parallel over batch (2) x tensor-parallel over heads
(4 groups of 4 heads). Each core computes q/k/v projections for its
head slice, causal attention for its 4 heads, and a partial c_proj
([2048,256] @ [256,1024]); the host sums the 4 partials per batch
(the "all-reduce") and folds the v/proj biases in at the end.

The reference uses scale = float32(C // (H ** -0.5)) = 4096.0 (exact
power of two), so logits are huge and softmax is near-one-hot; the
q/k projection and Q@K^T run in full fp32 to keep argmaxes faithful
(min top-2 scaled-logit gap on this data is ~0.2; fp16/bf16 logits
would flip rows). The value path (v projection, P@V, c_proj) runs in
fp16 — P entries are probabilities and v/out tolerances are ~5e-4.

K=64 fp32 matmuls waste half the PE array (measured 2.2x cost/row vs
K=128), so Q@K^T packs each HEAD PAIR into one K=128 matmul with a
block-diagonal stationary operand: lhsT[0:64, 0:64] = qT_h0 block,
lhsT[64:128, 64:128] = qT_h1 block, zeros elsewhere; rhs rows 0:64 =
kT_h0, 64:128 = kT_h1. Output rows 0:63 are h0's scores for 64
queries, rows 64:127 h1's — softmax stats stay row-wise. P@V runs
M=128 x N=128 (both heads' v side by side); the two diagonal [64,64]
quadrants of the product are the valid per-head outputs.

Per-core DRAM tensors:
  xT   [1024, 2048] f32  x[b] transposed (host-side)
  xTh  [1024, 2048] f16  same, half precision (for the v projection)
  wqk  [1024, 512]  f32  cols: [q_h0|q_h1|q_h2|q_h3|k_h0|k_h1|k_h2|k_h3]
  bqk  [512, 1]     f32  matching bias layout
  wv   [1024, 256]  f16  v columns for the head group
  wp   [256, 1024]  f16  w_proj rows for the head group
  outT [1024, 2048] f32  partial output, transposed
"""

import numpy as np

import concourse.bacc as bacc
import concourse.mybir as mybir
import concourse.tile as tile
from concourse.bass_utils import run_bass_kernel_spmd
from concourse.masks import make_identity, make_causal_mask

f32 = mybir.dt.float32
f16 = mybir.dt.float16
AF = mybir.ActivationFunctionType
AX = mybir.AxisListType

B, T, C = 2, 2048, 1024
H, HS = 16, 64
NCORES = 8
HG = 4            # head groups (cores per batch)
NHL = H // HG     # local heads per core = 4
P = 128
KT = C // P       # 8 contraction tiles
CH = 512          # free-dim chunk
NT = T // CH      # 4 token chunks
QB64 = T // HS    # 32 query blocks of 64
SCALE = 4096.0    # float32(C // (H ** -0.5)) — faithful to source bug
MASK_VAL = -1e10

_CACHE = {}


def _build_program():
    nc = bacc.Bacc("TRN2", target_bir_lowering=False, debug=False,
                   num_devices=NCORES)
    xTh = nc.dram_tensor("xTh", [C, T], f16, kind="ExternalInput").ap()
    xTl = nc.dram_tensor("xTl", [C, T], f16, kind="ExternalInput").ap()
    wqkh = nc.dram_tensor("wqkh", [C, 2 * NHL * HS], f16, kind="ExternalInput").ap()
    wqkl = nc.dram_tensor("wqkl", [C, 2 * NHL * HS], f16, kind="ExternalInput").ap()
    bqk = nc.dram_tensor("bqk", [2 * NHL * HS, 1], f32, kind="ExternalInput").ap()
    wv = nc.dram_tensor("wv", [C, NHL * HS], f16, kind="ExternalInput").ap()
    wp = nc.dram_tensor("wp", [NHL * HS, C], f16, kind="ExternalInput").ap()
    outT = nc.dram_tensor("outT", [C, T], f32, kind="ExternalOutput").ap()

    with tile.TileContext(nc) as tc:
        with (
            tc.tile_pool(name="const", bufs=1) as const,
            tc.tile_pool(name="wts", bufs=1) as wts,
            tc.tile_pool(name="xin", bufs=2) as xin,
            tc.tile_pool(name="qksb", bufs=1) as qksb,
            tc.tile_pool(name="vsb", bufs=1) as vsb,
            tc.tile_pool(name="att", bufs=3) as att,
            tc.tile_pool(name="aout", bufs=1) as aout,
            tc.tile_pool(name="stage", bufs=3) as stage,
            tc.tile_pool(name="ps_big", bufs=4, space="PSUM") as ps_big,
            tc.tile_pool(name="ps_med", bufs=1, space="PSUM") as ps_med,
            tc.tile_pool(name="ps_acc", bufs=2, space="PSUM") as ps_acc,
        ):
            ident = const.tile([P, P], f16, tag="ident")
            make_identity(nc, ident[:])
            # [128, 64] causal mask for 64-query blocks, both partition
            # halves identical (row r masks query position r % 64)
            mask = const.tile([P, HS], f32, tag="mask")
            make_causal_mask(nc, mask[0:HS, :], mask_val=MASK_VAL)
            nc.sync.dma_start(mask[HS:P, :], mask[0:HS, :])

            # ---- x chunk loads (chunk 0 hoisted before weights) ----
            def xload(nt):
                xhs, xls = [], []
                for kt in range(KT):
                    t_ = xin.tile([P, CH], f16, tag=f"xh{kt}", name=f"xh{kt}_{nt}")
                    nc.sync.dma_start(
                        t_[:], xTh[kt * P:(kt + 1) * P, nt * CH:(nt + 1) * CH])
                    xhs.append(t_)
                    t_ = xin.tile([P, CH], f16, tag=f"xl{kt}", name=f"xl{kt}_{nt}")
                    nc.sync.dma_start(
                        t_[:], xTl[kt * P:(kt + 1) * P, nt * CH:(nt + 1) * CH])
                    xls.append(t_)
                return xhs, xls

            x0 = xload(0)

            # ---- weight loads --------------------------------------
            wqkh_sb = []
            wqkl_sb = []
            wv_sb = []
            for kt in range(KT):
                t_ = wts.tile([P, 2 * NHL * HS], f16, tag=f"wqkh{kt}")
                nc.sync.dma_start(t_[:], wqkh[kt * P:(kt + 1) * P, :])
                wqkh_sb.append(t_)
                t_ = wts.tile([P, 2 * NHL * HS], f16, tag=f"wqkl{kt}")
                nc.sync.dma_start(t_[:], wqkl[kt * P:(kt + 1) * P, :])
                wqkl_sb.append(t_)
                t_ = wts.tile([P, NHL * HS], f16, tag=f"wv{kt}")
                nc.sync.dma_start(t_[:], wv[kt * P:(kt + 1) * P, :])
                wv_sb.append(t_)
            wp_sb = []

            def load_wp():
                for kt in range(NHL * HS // P):  # 2
                    t_ = wts.tile([P, C], f16, tag=f"wp{kt}", name=f"wp{kt}")
                    nc.sync.dma_start(t_[:], wp[kt * P:(kt + 1) * P, :])
                    wp_sb.append(t_)
            bqk_sb = []
            for mt in range(2 * NHL * HS // P):  # 4
                t_ = wts.tile([P, 1], f32, tag=f"bqk{mt}")
                nc.sync.dma_start(t_[:], bqk[mt * P:(mt + 1) * P, :])
                bqk_sb.append(t_)

            # ---- persistent activations ----------------------------
            # qbd[hp]: block-diagonal qT for head pair hp, [128, 2T]:
            #   col block 2g   rows 0:64   = qT_h(2hp),  tokens g*64..
            #   col block 2g+1 rows 64:128 = qT_h(2hp+1), tokens g*64..
            # kp[hp]: [128, T], rows 0:64 = kT_h(2hp), 64:128 = kT_h(2hp+1)
            qbd_h = [qksb.tile([P, 2 * T], f16, tag=f"qbdh{i}", name=f"qbdh{i}")
                     for i in range(2)]
            qbd_l = [qksb.tile([P, 2 * T], f16, tag=f"qbdl{i}", name=f"qbdl{i}")
                     for i in range(2)]
            kp_h = [qksb.tile([P, T], f16, tag=f"kph{i}", name=f"kph{i}")
                    for i in range(2)]
            kp_l = [qksb.tile([P, T], f16, tag=f"kpl{i}", name=f"kpl{i}")
                    for i in range(2)]
            v_sb = [vsb.tile([P, NHL * HS], f16, tag=f"v{i}", name=f"v{i}")
                    for i in range(T // P)]
            aout_sb = [aout.tile([P, T], f16, tag=f"at{i}", name=f"at{i}")
                       for i in range(2)]

            for t_ in qbd_h + qbd_l:
                nc.gpsimd.memset(t_[:], 0.0)

            # ---- phase 1: qkv projections (per token chunk) --------
            def phase1_chunk(nt):
                xhs, xls = x0 if nt == 0 else xload(nt)
                # q,k: exact-fp32 via fp16 hi/lo three-pass (hh, hl, lh)
                for mt in range(4):
                    ps = ps_big.tile([P, CH], f32, tag="big")
                    passes = [(wqkh_sb, xhs), (wqkh_sb, xls), (wqkl_sb, xhs)]
                    for pi, (wsb, xsb) in enumerate(passes):
                        for kt in range(KT):
                            nc.tensor.matmul(
                                ps[:], wsb[kt][:, mt * P:(mt + 1) * P],
                                xsb[kt][:],
                                start=(pi == 0 and kt == 0),
                                stop=(pi == 2 and kt == KT - 1))
                    if mt < 2:  # q: scatter into block-diagonal hi/lo
                        hp = mt
                        dh = qbd_h[hp][:, nt * 2 * CH:(nt + 1) * 2 * CH].rearrange(
                            "p (b t c) -> p b t c", t=2, c=HS)
                        dl = qbd_l[hp][:, nt * 2 * CH:(nt + 1) * 2 * CH].rearrange(
                            "p (b t c) -> p b t c", t=2, c=HS)
                        sv = ps[:].rearrange("p (b c) -> p b c", c=HS)
                        for half, sl in ((0, slice(0, HS)), (1, slice(HS, P))):
                            nc.vector.tensor_scalar_add(
                                dh[sl, :, half, :], sv[sl], bqk_sb[mt][sl])
                            nc.vector.scalar_tensor_tensor(
                                dl[sl, :, half, :], sv[sl], bqk_sb[mt][sl],
                                dh[sl, :, half, :],
                                op0=mybir.AluOpType.add,
                                op1=mybir.AluOpType.subtract)
                    else:  # k: plain pair layout, hi then lo
                        hp = mt - 2
                        cs = slice(nt * CH, (nt + 1) * CH)
                        nc.scalar.activation(
                            kp_h[hp][:, cs], ps[:], AF.Identity,
                            bias=bqk_sb[mt][:])
                        nc.vector.scalar_tensor_tensor(
                            kp_l[hp][:, cs], ps[:], bqk_sb[mt][:],
                            kp_h[hp][:, cs],
                            op0=mybir.AluOpType.add,
                            op1=mybir.AluOpType.subtract)
                # v (natural layout, fp16): lhsT = xTh block, rhs = wv
                for tt in range(CH // P):
                    ps = ps_med.tile([P, NHL * HS], f32, tag="med")
                    for kt in range(KT):
                        nc.tensor.matmul(
                            ps[:], xhs[kt][:, tt * P:(tt + 1) * P], wv_sb[kt][:],
                            start=(kt == 0), stop=(kt == KT - 1))
                    nc.vector.tensor_copy(v_sb[nt * (CH // P) + tt][:], ps[:])

            # ---- phase 2: causal attention, software-pipelined -----
            # PE engine queues are in-order: emit S(it) before PT/PV(it-1)
            # so the PE never idles waiting on exp(it-1) (ACT).
            def proj_chunk(nt):
                for mt in range(C // P):
                    ps = ps_big.tile([P, CH], f32, tag="big",
                                     name=f"proj_{mt}_{nt}")
                    for kt in range(2):
                        nc.tensor.matmul(
                            ps[:], wp_sb[kt][:, mt * P:(mt + 1) * P],
                            aout_sb[kt][:, nt * CH:(nt + 1) * CH],
                            start=(kt == 0), stop=(kt == 1))
                    st = stage.tile([P, CH], f32, tag="stage",
                                    name=f"stg_{mt}_{nt}")
                    nc.scalar.activation(st[:], ps[:], AF.Copy)
                    nc.sync.dma_start(
                        outT[mt * P:(mt + 1) * P, nt * CH:(nt + 1) * CH], st[:])

            def s_stage(qb, hp):
                klen = (qb + 1) * HS
                nch = (klen + CH - 1) // CH
                s_chunks = []
                mx = att.tile([P, 4], f32, tag="mx", name=f"mx_{qb}_{hp}")
                for kc in range(nch):
                    w = min(CH, klen - kc * CH)
                    ps = ps_big.tile([P, CH], f32, tag="big",
                                     name=f"s_{qb}_{hp}_{kc}")
                    qs = slice(qb * P, (qb + 1) * P)
                    ks = slice(kc * CH, kc * CH + w)
                    for pi, (qt_, kt_) in enumerate((
                            (qbd_h[hp], kp_h[hp]), (qbd_h[hp], kp_l[hp]),
                            (qbd_l[hp], kp_h[hp]))):
                        nc.tensor.matmul(
                            ps[:, :w], qt_[:, qs], kt_[:, ks],
                            start=(pi == 0), stop=(pi == 2))
                    if kc == nch - 1:  # diag 64-block is last valid cols
                        off = klen - kc * CH - HS
                        nc.vector.tensor_add(
                            ps[:, off:off + HS], ps[:, off:off + HS], mask[:])
                    nc.vector.reduce_max(mx[:, kc:kc + 1], ps[:, :w], axis=AX.X)
                    s_chunks.append((ps, w))
                nm = att.tile([P, 1], f32, tag="nm", name=f"nm_{qb}_{hp}")
                nc.vector.reduce_max(nm[:], mx[:, :nch], axis=AX.X, negate=True)
                nmb = att.tile([P, 1], f32, tag="nmb", name=f"nmb_{qb}_{hp}")
                nc.vector.tensor_scalar_mul(nmb[:], nm[:], SCALE)
                p_sb = att.tile([P, T], f16, tag="P", name=f"p_{qb}_{hp}")
                lp = att.tile([P, 4], f32, tag="lp", name=f"lp_{qb}_{hp}")
                for kc, (ps, w) in enumerate(s_chunks):
                    nc.scalar.activation(
                        p_sb[:, kc * CH:kc * CH + w], ps[:, :w], AF.Exp,
                        bias=nmb[:], scale=SCALE,
                        accum_out=lp[:, kc:kc + 1])
                l_ = att.tile([P, 1], f32, tag="l", name=f"l_{qb}_{hp}")
                nc.vector.reduce_sum(l_[:], lp[:, :nch], axis=AX.X)
                linv = att.tile([P, 1], f32, tag="linv", name=f"li_{qb}_{hp}")
                nc.vector.reciprocal(linv[:], l_[:])
                return dict(qb=qb, hp=hp, klen=klen, p_sb=p_sb, linv=linv)

            def pv_stage(st_):
                qb, hp = st_["qb"], st_["hp"]
                klen, p_sb, linv = st_["klen"], st_["p_sb"], st_["linv"]
                o_ps = ps_acc.tile([P, P], f32, tag="acc",
                                   name=f"o_{qb}_{hp}")
                nkb = (klen + P - 1) // P
                for kc in range((nkb + 3) // 4):
                    jmax = min(4, nkb - kc * 4)
                    pt_ps = ps_med.tile([P, CH], f16, tag="med",
                                        name=f"ptp_{qb}_{hp}_{kc}")
                    wlast = P
                    for j in range(jmax):
                        kb = kc * 4 + j
                        kw = min(P, klen - kb * P)
                        nc.tensor.transpose(
                            pt_ps[0:kw, j * P:(j + 1) * P],
                            p_sb[:, kb * P:kb * P + kw], ident[:])
                        wlast = kw
                    pt_sb = att.tile([P, CH], f16, tag="pts",
                                     name=f"pts_{qb}_{hp}_{kc}")
                    nfull = jmax - (1 if wlast < P else 0)
                    if nfull:
                        nc.vector.tensor_copy(
                            pt_sb[:, :nfull * P], pt_ps[:, :nfull * P])
                    if wlast < P:
                        nc.vector.tensor_copy(
                            pt_sb[0:wlast, nfull * P:(nfull + 1) * P],
                            pt_ps[0:wlast, nfull * P:(nfull + 1) * P])
                        nc.gpsimd.memset(
                            pt_sb[wlast:P, nfull * P:(nfull + 1) * P], 0.0)
                    for j in range(jmax):
                        kb = kc * 4 + j
                        nc.tensor.matmul(
                            o_ps[:], pt_sb[:, j * P:(j + 1) * P],
                            v_sb[kb][:, hp * P:(hp + 1) * P],
                            start=(kb == 0), stop=(kb == nkb - 1))
                ao = att.tile([P, HS], f16, tag="ao", name=f"ao_{qb}_{hp}")
                nc.vector.tensor_scalar_mul(ao[0:HS, :], o_ps[0:HS, 0:HS],
                                            linv[0:HS])
                nc.vector.tensor_scalar_mul(ao[HS:P, :], o_ps[HS:P, HS:P],
                                            linv[HS:P])
                at_ps = ps_med.tile([P, HS], f16, tag="med",
                                    name=f"at_{qb}_{hp}")
                nc.tensor.transpose(at_ps[0:HS, :], ao[0:HS, :],
                                    ident[0:HS, 0:HS])
                nc.tensor.matmul(at_ps[HS:P, :], ao[HS:P, :],
                                 ident[HS:P, HS:P], is_transpose=True,
                                 skip_group_check=True)
                nc.vector.tensor_copy(
                    aout_sb[hp][:, qb * HS:(qb + 1) * HS], at_ps[:])
                if hp == 1 and (qb + 1) % (CH // HS) == 0:
                    proj_chunk((qb + 1) // (CH // HS) - 1)

            pending = None
            nqb = QB64 // NT  # 8 query blocks unlocked per token chunk
            for nt in range(NT):
                phase1_chunk(nt)
                if nt == 0:
                    load_wp()
                for qb in range(nt * nqb, (nt + 1) * nqb):
                    for hp in range(2):
                        st_ = s_stage(qb, hp)
                        if pending is not None:
                            pv_stage(pending)
                        pending = st_
            pv_stage(pending)

    nc.compile()
    return nc


def _get_program():
    if "nc" not in _CACHE:
        _CACHE["nc"] = _build_program()
    return _CACHE["nc"]


def _per_core_inputs(x, w_attn, b_attn, w_proj):
    in_maps = []
    for core in range(NCORES):
        b = core // HG
        hg = core % HG
        xTc = np.ascontiguousarray(x[b].T.astype(np.float32))
        xh = xTc.astype(np.float16)
        xl = (xTc - xh.astype(np.float32)).astype(np.float16)
        qcols = []
        bcols = []
        # q head-pairs then k head-pairs: [q01 | q23 | k01 | k23]
        for off in (0, C):  # q then k
            for j in range(NHL):
                hgl = hg * NHL + j
                qcols.append(w_attn[:, off + hgl * HS: off + (hgl + 1) * HS])
                bcols.append(b_attn[off + hgl * HS: off + (hgl + 1) * HS])
        wqk_ = np.ascontiguousarray(
            np.concatenate(qcols, axis=1).astype(np.float32))
        wqkh_ = wqk_.astype(np.float16)
        wqkl_ = (wqk_ - wqkh_.astype(np.float32)).astype(np.float16)
        bqk_ = np.ascontiguousarray(
            np.concatenate(bcols)[:, None].astype(np.float32))
        wv_ = np.ascontiguousarray(
            w_attn[:, 2 * C + hg * NHL * HS: 2 * C + (hg + 1) * NHL * HS]
            .astype(np.float16))
        wp_ = np.ascontiguousarray(
            w_proj[hg * NHL * HS:(hg + 1) * NHL * HS, :].astype(np.float16))
        in_maps.append({"xTh": xh, "xTl": xl, "wqkh": wqkh_, "wqkl": wqkl_,
                        "bqk": bqk_, "wv": wv_, "wp": wp_})
    return in_maps


def run_sharded(x, w_attn, b_attn, w_proj, b_proj, trace=False, **kw):
    nc = _get_program()
    in_maps = _per_core_inputs(x, w_attn, b_attn, w_proj)
    res = run_bass_kernel_spmd(nc, in_maps, core_ids=list(range(NCORES)),
                               trace=trace, **kw)
    out = np.zeros((B, T, C), dtype=np.float32)
    for core in range(NCORES):
        out[core // HG] += res.results[core]["outT"].T
    corr = (b_attn[2 * C:].astype(np.float32) @ w_proj.astype(np.float32)
            + b_proj.astype(np.float32))
    out += corr[None, None, :]
    return out, res


def kernel(x, w_attn, b_attn, w_proj, b_proj):
    out, _ = run_sharded(np.asarray(x), np.asarray(w_attn), np.asarray(b_attn),
                         np.asarray(w_proj), np.asarray(b_proj))
    return out


# revision 18
# speedup vs baseline: 1.1118x; 1.1118x over previous
"""Causal self-attention (B=2, T=2048, C=1024, H=16) on 8 NeuronCores.

Sharding: data-parallel over batch (2) x tensor-parallel over heads
(4 groups of 4 heads). Each core computes q/k/v projections for its
head slice, causal attention for its 4 heads, and a partial c_proj
([2048,256] @ [256,1024]); the host sums the 4 partials per batch
(the "all-reduce") and folds the v/proj biases in at the end.

The reference uses scale = float32(C // (H ** -0.5)) = 4096.0 (exact
power of two), so logits are huge and softmax is near-one-hot; the
q/k projection and Q@K^T run in full fp32 to keep argmaxes faithful
(min top-2 scaled-logit gap on this data is ~0.2; fp16/bf16 logits
would flip rows). The value path (v projection, P@V, c_proj) runs in
fp16 — P entries are probabilities and v/out tolerances are ~5e-4.

K=64 fp32 matmuls waste half the PE array (measured 2.2x cost/row vs
K=128), so Q@K^T packs each HEAD PAIR into one K=128 matmul with a
block-diagonal stationary operand: lhsT[0:64, 0:64] = qT_h0 block,
lhsT[64:128, 64:128] = qT_h1 block, zeros elsewhere; rhs rows 0:64 =
kT_h0, 64:128 = kT_h1. Output rows 0:63 are h0's scores for 64
queries, rows 64:127 h1's — softmax stats stay row-wise. P@V runs
M=128 x N=128 (both heads' v side by side); the two diagonal [64,64]
quadrants of the product are the valid per-head outputs.

Per-core DRAM tensors:
  xT   [1024, 2048] f32  x[b] transposed (host-side)
  xTh  [1024, 2048] f16  same, half precision (for the v projection)
  wqk  [1024, 512]  f32  cols: [q_h0|q_h1|q_h2|q_h3|k_h0|k_h1|k_h2|k_h3]
  bqk  [512, 1]     f32  matching bias layout
  wv   [1024, 256]  f16  v columns for the head group
  wp   [256, 1024]  f16  w_proj rows for the head group
  outT [1024, 2048] f32  partial output, transposed
"""

import numpy as np

import concourse.bacc as bacc
import concourse.mybir as mybir
import concourse.tile as tile
from concourse.bass_utils import run_bass_kernel_spmd
from concourse.masks import make_identity, make_causal_mask

f32 = mybir.dt.float32
f16 = mybir.dt.float16
AF = mybir.ActivationFunctionType
AX = mybir.AxisListType

B, T, C = 2, 2048, 1024
H, HS = 16, 64
NCORES = 8
HG = 4            # head groups (cores per batch)
NHL = H // HG     # local heads per core = 4
P = 128
KT = C // P       # 8 contraction tiles
CH = 512          # free-dim chunk
NT = T // CH      # 4 token chunks
QB64 = T // HS    # 32 query blocks of 64
SCALE = 4096.0    # float32(C // (H ** -0.5)) — faithful to source bug
MASK_VAL = -1e10

_CACHE = {}


def _build_program():
    nc = bacc.Bacc("TRN2", target_bir_lowering=False, debug=False,
                   num_devices=NCORES)
    xTh = nc.dram_tensor("xTh", [C, T], f16, kind="ExternalInput").ap()
    xTl = nc.dram_tensor("xTl", [C, T], f16, kind="ExternalInput").ap()
    wqkh = nc.dram_tensor("wqkh", [C, 2 * NHL * HS], f16, kind="ExternalInput").ap()
    wqkl = nc.dram_tensor("wqkl", [C, 2 * NHL * HS], f16, kind="ExternalInput").ap()
    bqk = nc.dram_tensor("bqk", [2 * NHL * HS, 1], f32, kind="ExternalInput").ap()
    wv = nc.dram_tensor("wv", [C, NHL * HS], f16, kind="ExternalInput").ap()
    wp = nc.dram_tensor("wp", [NHL * HS, C], f16, kind="ExternalInput").ap()
    outT = nc.dram_tensor("outT", [C, T], f32, kind="ExternalOutput").ap()

    with tile.TileContext(nc) as tc:
        with (
            tc.tile_pool(name="const", bufs=1) as const,
            tc.tile_pool(name="wts", bufs=1) as wts,
            tc.tile_pool(name="xin", bufs=2) as xin,
            tc.tile_pool(name="qksb", bufs=1) as qksb,
            tc.tile_pool(name="vsb", bufs=1) as vsb,
            tc.tile_pool(name="att", bufs=3) as att,
            tc.tile_pool(name="aout", bufs=1) as aout,
            tc.tile_pool(name="stage", bufs=3) as stage,
            tc.tile_pool(name="ps_big", bufs=4, space="PSUM") as ps_big,
            tc.tile_pool(name="ps_med", bufs=2, space="PSUM") as ps_med,
            tc.tile_pool(name="ps_acc", bufs=2, space="PSUM") as ps_acc,
        ):
            ident = const.tile([P, P], f16, tag="ident")
            make_identity(nc, ident[:])
            # [128, 64] causal mask for 64-query blocks, both partition
            # halves identical (row r masks query position r % 64)
            mask = const.tile([P, HS], f32, tag="mask")
            make_causal_mask(nc, mask[0:HS, :], mask_val=MASK_VAL)
            nc.sync.dma_start(mask[HS:P, :], mask[0:HS, :])

            # ---- x chunk loads (chunk 0 hoisted before weights) ----
            def xload(nt):
                xhs, xls = [], []
                for kt in range(KT):
                    t_ = xin.tile([P, CH], f16, tag=f"xh{kt}", name=f"xh{kt}_{nt}")
                    nc.sync.dma_start(
                        t_[:], xTh[kt * P:(kt + 1) * P, nt * CH:(nt + 1) * CH])
                    xhs.append(t_)
                    t_ = xin.tile([P, CH], f16, tag=f"xl{kt}", name=f"xl{kt}_{nt}")
                    nc.sync.dma_start(
                        t_[:], xTl[kt * P:(kt + 1) * P, nt * CH:(nt + 1) * CH])
                    xls.append(t_)
                return xhs, xls

            x0 = xload(0)

            # ---- weight loads --------------------------------------
            wqkh_sb = []
            wqkl_sb = []
            wv_sb = []
            for kt in range(KT):
                t_ = wts.tile([P, 2 * NHL * HS], f16, tag=f"wqkh{kt}")
                nc.sync.dma_start(t_[:], wqkh[kt * P:(kt + 1) * P, :])
                wqkh_sb.append(t_)
                t_ = wts.tile([P, 2 * NHL * HS], f16, tag=f"wqkl{kt}")
                nc.sync.dma_start(t_[:], wqkl[kt * P:(kt + 1) * P, :])
                wqkl_sb.append(t_)
                t_ = wts.tile([P, NHL * HS], f16, tag=f"wv{kt}")
                nc.sync.dma_start(t_[:], wv[kt * P:(kt + 1) * P, :])
                wv_sb.append(t_)
            wp_sb = []

            def load_wp():
                for kt in range(NHL * HS // P):  # 2
                    t_ = wts.tile([P, C], f16, tag=f"wp{kt}", name=f"wp{kt}")
                    nc.sync.dma_start(t_[:], wp[kt * P:(kt + 1) * P, :])
                    wp_sb.append(t_)
            bqk_sb = []
            for mt in range(2 * NHL * HS // P):  # 4
                t_ = wts.tile([P, 1], f32, tag=f"bqk{mt}")
                nc.sync.dma_start(t_[:], bqk[mt * P:(mt + 1) * P, :])
                bqk_sb.append(t_)

            # ---- persistent activations ----------------------------
            # qbd[hp]: block-diagonal qT for head pair hp, [128, 2T]:
            #   col block 2g   rows 0:64   = qT_h(2hp),  tokens g*64..
            #   col block 2g+1 rows 64:128 = qT_h(2hp+1), tokens g*64..
            # kp[hp]: [128, T], rows 0:64 = kT_h(2hp), 64:128 = kT_h(2hp+1)
            qbd_h = [qksb.tile([P, 2 * T], f16, tag=f"qbdh{i}", name=f"qbdh{i}")
                     for i in range(2)]
            qbd_l = [qksb.tile([P, 2 * T], f16, tag=f"qbdl{i}", name=f"qbdl{i}")
                     for i in range(2)]
            kp_h = [qksb.tile([P, T], f16, tag=f"kph{i}", name=f"kph{i}")
                    for i in range(2)]
            kp_l = [qksb.tile([P, T], f16, tag=f"kpl{i}", name=f"kpl{i}")
                    for i in range(2)]
            v_sb = [vsb.tile([P, NHL * HS], f16, tag=f"v{i}", name=f"v{i}")
                    for i in range(T // P)]
            aout_sb = [aout.tile([P, T], f16, tag=f"at{i}", name=f"at{i}")
                       for i in range(2)]

            for t_ in qbd_h + qbd_l:
                nc.gpsimd.memset(t_[:], 0.0)

            # ---- phase 1: qkv projections (per token chunk) --------
            def phase1_chunk(nt):
                xhs, xls = x0 if nt == 0 else xload(nt)
                # q,k: exact-fp32 via fp16 hi/lo three-pass (hh, hl, lh)
                for mt in range(4):
                    ps = ps_big.tile([P, CH], f32, tag="big")
                    passes = [(wqkh_sb, xhs), (wqkh_sb, xls), (wqkl_sb, xhs)]
                    for pi, (wsb, xsb) in enumerate(passes):
                        for kt in range(KT):
                            nc.tensor.matmul(
                                ps[:], wsb[kt][:, mt * P:(mt + 1) * P],
                                xsb[kt][:],
                                start=(pi == 0 and kt == 0),
                                stop=(pi == 2 and kt == KT - 1))
                    if mt < 2:  # q: scatter into block-diagonal hi/lo
                        hp = mt
                        dh = qbd_h[hp][:, nt * 2 * CH:(nt + 1) * 2 * CH].rearrange(
                            "p (b t c) -> p b t c", t=2, c=HS)
                        dl = qbd_l[hp][:, nt * 2 * CH:(nt + 1) * 2 * CH].rearrange(
                            "p (b t c) -> p b t c", t=2, c=HS)
                        sv = ps[:].rearrange("p (b c) -> p b c", c=HS)
                        for half, sl in ((0, slice(0, HS)), (1, slice(HS, P))):
                            nc.vector.tensor_scalar_add(
                                dh[sl, :, half, :], sv[sl], bqk_sb[mt][sl])
                            nc.vector.scalar_tensor_tensor(
                                dl[sl, :, half, :], sv[sl], bqk_sb[mt][sl],
                                dh[sl, :, half, :],
                                op0=mybir.AluOpType.add,
                                op1=mybir.AluOpType.subtract)
                    else:  # k: plain pair layout, hi then lo
                        hp = mt - 2
                        cs = slice(nt * CH, (nt + 1) * CH)
                        nc.scalar.activation(
                            kp_h[hp][:, cs], ps[:], AF.Identity,
                            bias=bqk_sb[mt][:])
                        nc.vector.scalar_tensor_tensor(
                            kp_l[hp][:, cs], ps[:], bqk_sb[mt][:],
                            kp_h[hp][:, cs],
                            op0=mybir.AluOpType.add,
                            op1=mybir.AluOpType.subtract)
                # v (natural layout, fp16): lhsT = xTh block, rhs = wv
                for tt in range(CH // P):
                    ps = ps_med.tile([P, NHL * HS], f32, tag="med")
                    for kt in range(KT):
                        nc.tensor.matmul(
                            ps[:], xhs[kt][:, tt * P:(tt + 1) * P], wv_sb[kt][:],
                            start=(kt == 0), stop=(kt == KT - 1))
                    nc.vector.tensor_copy(v_sb[nt * (CH // P) + tt][:], ps[:])

            # ---- phase 2: causal attention, software-pipelined -----
            # PE engine queues are in-order: emit S(it) before PT/PV(it-1)
            # so the PE never idles waiting on exp(it-1) (ACT).
            def proj_chunk(nt):
                for mt in range(C // P):
                    ps = ps_big.tile([P, CH], f32, tag="big",
                                     name=f"proj_{mt}_{nt}")
                    for kt in range(2):
                        nc.tensor.matmul(
                            ps[:], wp_sb[kt][:, mt * P:(mt + 1) * P],
                            aout_sb[kt][:, nt * CH:(nt + 1) * CH],
                            start=(kt == 0), stop=(kt == 1))
                    st = stage.tile([P, CH], f32, tag="stage",
                                    name=f"stg_{mt}_{nt}")
                    nc.scalar.activation(st[:], ps[:], AF.Copy)
                    nc.sync.dma_start(
                        outT[mt * P:(mt + 1) * P, nt * CH:(nt + 1) * CH], st[:])

            def s_stage(qb, hp):
                klen = (qb + 1) * HS
                nch = (klen + CH - 1) // CH
                s_chunks = []
                mx = att.tile([P, 4], f32, tag="mx", name=f"mx_{qb}_{hp}")
                for kc in range(nch):
                    w = min(CH, klen - kc * CH)
                    ps = ps_big.tile([P, CH], f32, tag="big",
                                     name=f"s_{qb}_{hp}_{kc}")
                    qs = slice(qb * P, (qb + 1) * P)
                    ks = slice(kc * CH, kc * CH + w)
                    for pi, (qt_, kt_) in enumerate((
                            (qbd_h[hp], kp_h[hp]), (qbd_h[hp], kp_l[hp]),
                            (qbd_l[hp], kp_h[hp]))):
                        nc.tensor.matmul(
                            ps[:, :w], qt_[:, qs], kt_[:, ks],
                            start=(pi == 0), stop=(pi == 2))
                    if kc == nch - 1:  # diag 64-block is last valid cols
                        off = klen - kc * CH - HS
                        nc.vector.tensor_add(
                            ps[:, off:off + HS], ps[:, off:off + HS], mask[:])
                    nc.vector.reduce_max(mx[:, kc:kc + 1], ps[:, :w], axis=AX.X)
                    s_chunks.append((ps, w))
                nm = att.tile([P, 1], f32, tag="nm", name=f"nm_{qb}_{hp}")
                nc.vector.reduce_max(nm[:], mx[:, :nch], axis=AX.X, negate=True)
                nmb = att.tile([P, 1], f32, tag="nmb", name=f"nmb_{qb}_{hp}")
                nc.vector.tensor_scalar_mul(nmb[:], nm[:], SCALE)
                p_sb = att.tile([P, T], f16, tag="P", name=f"p_{qb}_{hp}")
                lp = att.tile([P, 4], f32, tag="lp", name=f"lp_{qb}_{hp}")
                for kc, (ps, w) in enumerate(s_chunks):
                    nc.scalar.activation(
                        p_sb[:, kc * CH:kc * CH + w], ps[:, :w], AF.Exp,
                        bias=nmb[:], scale=SCALE,
                        accum_out=lp[:, kc:kc + 1])
                l_ = att.tile([P, 1], f32, tag="l", name=f"l_{qb}_{hp}")
                nc.vector.reduce_sum(l_[:], lp[:, :nch], axis=AX.X)
                linv = att.tile([P, 1], f32, tag="linv", name=f"li_{qb}_{hp}")
                nc.vector.reciprocal(linv[:], l_[:])
                return dict(qb=qb, hp=hp, klen=klen, p_sb=p_sb, linv=linv)

            def pv_stage(st_):
                qb, hp = st_["qb"], st_["hp"]
                klen, p_sb, linv = st_["klen"], st_["p_sb"], st_["linv"]
                o_ps = ps_acc.tile([P, P], f32, tag="acc",
                                   name=f"o_{qb}_{hp}")
                nkb = (klen + P - 1) // P
                for kc in range((nkb + 3) // 4):
                    jmax = min(4, nkb - kc * 4)
                    pt_ps = ps_med.tile([P, CH], f16, tag="med",
                                        name=f"ptp_{qb}_{hp}_{kc}")
                    wlast = P
                    for j in range(jmax):
                        kb = kc * 4 + j
                        kw = min(P, klen - kb * P)
                        nc.tensor.transpose(
                            pt_ps[0:kw, j * P:(j + 1) * P],
                            p_sb[:, kb * P:kb * P + kw], ident[:])
                        wlast = kw
                    pt_sb = att.tile([P, CH], f16, tag="pts",
                                     name=f"pts_{qb}_{hp}_{kc}")
                    nfull = jmax - (1 if wlast < P else 0)
                    if nfull:
                        nc.vector.tensor_copy(
                            pt_sb[:, :nfull * P], pt_ps[:, :nfull * P])
                    if wlast < P:
                        nc.vector.tensor_copy(
                            pt_sb[0:wlast, nfull * P:(nfull + 1) * P],
                            pt_ps[0:wlast, nfull * P:(nfull + 1) * P])
                        nc.gpsimd.memset(
                            pt_sb[wlast:P, nfull * P:(nfull + 1) * P], 0.0)
                    for j in range(jmax):
                        kb = kc * 4 + j
                        nc.tensor.matmul(
                            o_ps[:], pt_sb[:, j * P:(j + 1) * P],
                            v_sb[kb][:, hp * P:(hp + 1) * P],
                            start=(kb == 0), stop=(kb == nkb - 1))
                ao = att.tile([P, HS], f16, tag="ao", name=f"ao_{qb}_{hp}")
                nc.vector.tensor_scalar_mul(ao[0:HS, :], o_ps[0:HS, 0:HS],
                                            linv[0:HS])
                nc.vector.tensor_scalar_mul(ao[HS:P, :], o_ps[HS:P, HS:P],
                                            linv[HS:P])
                at_ps = ps_med.tile([P, HS], f16, tag="med",
                                    name=f"at_{qb}_{hp}")
                nc.tensor.transpose(at_ps[0:HS, :], ao[0:HS, :],
                                    ident[0:HS, 0:HS])
                nc.tensor.matmul(at_ps[HS:P, :], ao[HS:P, :],
                                 ident[HS:P, HS:P], is_transpose=True,
                                 skip_group_check=True)
                nc.vector.tensor_copy(
                    aout_sb[hp][:, qb * HS:(qb + 1) * HS], at_ps[:])
                if hp == 1 and (qb + 1) % (CH // HS) == 0:
                    proj_chunk((qb + 1) // (CH // HS) - 1)

            from collections import deque
            pend = deque()
            nqb = QB64 // NT  # 8 query blocks unlocked per token chunk
            for nt in range(NT):
                phase1_chunk(nt)
                if nt == 0:
                    load_wp()
                for qb in range(nt * nqb, (nt + 1) * nqb):
                    for hp in range(2):
                        pend.append(s_stage(qb, hp))
                        if len(pend) > 2:
                            pv_stage(pend.popleft())
            while pend:
                pv_stage(pend.popleft())

    nc.compile()
    return nc


def _get_program():
    if "nc" not in _CACHE:
        _CACHE["nc"] = _build_program()
    return _CACHE["nc"]


def _per_core_inputs(x, w_attn, b_attn, w_proj):
    in_maps = []
    for core in range(NCORES):
        b = core // HG
        hg = core % HG
        xTc = np.ascontiguousarray(x[b].T.astype(np.float32))
        xh = xTc.astype(np.float16)
        xl = (xTc - xh.astype(np.float32)).astype(np.float16)
        qcols = []
        bcols = []
        # q head-pairs then k head-pairs: [q01 | q23 | k01 | k23]
        for off in (0, C):  # q then k
            for j in range(NHL):
                hgl = hg * NHL + j
                qcols.append(w_attn[:, off + hgl * HS: off + (hgl + 1) * HS])
                bcols.append(b_attn[off + hgl * HS: off + (hgl + 1) * HS])
        wqk_ = np.ascontiguousarray(
            np.concatenate(qcols, axis=1).astype(np.float32))
        wqkh_ = wqk_.astype(np.float16)
        wqkl_ = (wqk_ - wqkh_.astype(np.float32)).astype(np.float16)
        bqk_ = np.ascontiguousarray(
            np.concatenate(bcols)[:, None].astype(np.float32))
        wv_ = np.ascontiguousarray(
            w_attn[:, 2 * C + hg * NHL * HS: 2 * C + (hg + 1) * NHL * HS]
            .astype(np.float16))
        wp_ = np.ascontiguousarray(
            w_proj[hg * NHL * HS:(hg + 1) * NHL * HS, :].astype(np.float16))
        in_maps.append({"xTh": xh, "xTl": xl, "wqkh": wqkh_, "wqkl": wqkl_,
                        "bqk": bqk_, "wv": wv_, "wp": wp_})
    return in_maps


def run_sharded(x, w_attn, b_attn, w_proj, b_proj, trace=False, **kw):
    nc = _get_program()
    in_maps = _per_core_inputs(x, w_attn, b_attn, w_proj)
    res = run_bass_kernel_spmd(nc, in_maps, core_ids=list(range(NCORES)),
                               trace=trace, **kw)
    out = np.zeros((B, T, C), dtype=np.float32)
    for core in range(NCORES):
        out[core // HG] += res.results[core]["outT"].T
    corr = (b_attn[2 * C:].astype(np.float32) @ w_proj.astype(np.float32)
            + b_proj.astype(np.float32))
    out += corr[None, None, :]
    return out, res


def kernel(x, w_attn, b_attn, w_proj, b_proj):
    out, _ = run_sharded(np.asarray(x), np.asarray(w_attn), np.asarray(b_attn),
                         np.asarray(w_proj), np.asarray(b_proj))
    return out
